# revision 8
# baseline (speedup 1.0000x reference)
"""Controlled-unitary gate (24 wires) on 8 trn2 NeuronCores.

Math: the reference's wire-permutation is pure index bookkeeping that cancels
in the flat layout.  With x4 = x.reshape(2,2,1024,4096) (control bit, b1,
target bits, low bits), the whole op is

    y4[k, b1] = expm(S_k) @ x4[k, b1],      S_k = A_k - A_k^H  (skew-Hermitian)

Device algorithm (per core; cores 0-3 handle control 0, cores 4-7 control 1;
within a group each core owns a 256-row shard of the 1024x1024 matrices and a
1024-column shard of the state):

  expm via scaling-and-squaring: A0 = S/2^6, degree-14 Taylor in
  Paterson-Stockmeyer form (powers A2, A3; Horner over A3), then 6 squarings.
  All expm matmuls run as bf16 hi/lo split products (3 passes per real
  product) accumulating in fp32 PSUM -> ~1e-4 relative error.
  Row-sharded chain: lhsT = transpose of the local shard (PE transpose),
  rhs = full matrix (replicated via AllGather inside each 4-core group).
  The final state multiply uses fp32r (FP22) matmuls at full PE rate.
"""

import os
from contextlib import ExitStack
from math import factorial

import numpy as np
import ml_dtypes

import concourse.bass as bass
import concourse.tile as tile
import concourse.mybir as mybir
from concourse import bacc
from concourse.bass_utils import run_bass_kernel_spmd

F32 = mybir.dt.float32
F32R = mybir.dt.float32r
BF16 = mybir.dt.bfloat16

D = 1024            # target-space dim
P = 128             # partitions
KT = D // P         # 8 k-tiles
SH = D // 4         # 256 rows per core shard
MT = SH // P        # 2 m-tiles per shard
S_SQ = 6            # squarings
M_TAY = 14          # Taylor degree (Paterson-Stockmeyer p=3)
CE = [1.0 / factorial(k) for k in range(M_TAY + 1)]
SCALE = 1.0 / (1 << S_SQ)
XB = 2              # b1 values
XC = 1024           # state columns per core per b1
NCORES = 8
GROUP = 4
RG = [[0, 1, 2, 3], [4, 5, 6, 7]]

# split-product pass lists: out_r += Zr*Rr - Zi*Ri ; out_i += Zr*Ri + Zi*Rr
# lhsT keys: rh/rl (Zr^T hi/lo), ih/il (Zi^T), nih/nil (-Zi^T)
_PASSES_R = [("rh", "rh"), ("rh", "rl"), ("rl", "rh"),
             ("nih", "ih"), ("nih", "il"), ("nil", "ih")]
_PASSES_I = [("rh", "ih"), ("rh", "il"), ("rl", "ih"),
             ("ih", "rh"), ("ih", "rl"), ("il", "rh")]


def _emit(nc, tc, t_in, t_out, pools):
    """Emit the whole per-core program (rank-oblivious)."""
    ps_pool, pst_pool, io_pool = pools["ps"], pools["pst"], pools["io"]

    idn = io_pool.tile([P, P], BF16, tag="idn")
    idn32 = io_pool.tile([P, P], F32, tag="idn32")
    nc.sync.dma_start(idn[:], t_in["idn"].ap())
    nc.sync.dma_start(idn32[:], t_in["idn32"].ap())

    # ---------------- helpers -------------------------------------------
    def make_lhsT(pool, cur, gen):
        """cur: dict rh/rl/ih/il -> AP [P, MT, D] (bf16 row shard planes).
        Returns lhsT dict of [P, KT, SH] planes (plus negated imag)."""
        lt = {}
        for key in ("rh", "rl", "ih", "il"):
            t = pool.tile([P, KT, SH], BF16, tag=f"lt_{key}", bufs=2,
                          name=f"lt_{key}_{gen}")
            for mt in range(MT):
                for ct in range(KT):
                    pt = pst_pool.tile([P, P], BF16, tag="pst",
                                       name=f"pt_{gen}_{key}_{mt}_{ct}")
                    nc.tensor.transpose(pt[:], cur[key][:, mt, ct * P:(ct + 1) * P],
                                        idn[:])
                    nc.vector.tensor_copy(t[:, ct, mt * P:(mt + 1) * P], pt[:])
            lt[key] = t
        for src, dst in (("ih", "nih"), ("il", "nil")):
            t = pool.tile([P, KT, SH], BF16, tag=f"lt_{dst}", bufs=2,
                          name=f"lt_{dst}_{gen}")
            nc.vector.tensor_scalar_mul(t[:], lt[src][:], -1.0)
            lt[dst] = t
        return lt

    def emit_cmm_banks(lhsT, rhs, gen):
        """Yield (nch, mt, ps_r, ps_i) PSUM banks of the complex product
        (shard @ full): out rows SH, cols D, chunked by 512."""
        for nch in range(D // 512):
            csl = slice(nch * 512, (nch + 1) * 512)
            for mt in range(MT):
                ps_r = ps_pool.tile([P, 512], F32, tag="ps",
                                    name=f"psr_{gen}_{nch}_{mt}")
                ps_i = ps_pool.tile([P, 512], F32, tag="ps",
                                    name=f"psi_{gen}_{nch}_{mt}")
                for bank, passes in ((ps_r, _PASSES_R), (ps_i, _PASSES_I)):
                    n = len(passes) * KT
                    idx = 0
                    for lp, rp in passes:
                        for kt in range(KT):
                            nc.tensor.matmul(
                                bank[:],
                                lhsT[lp][:, kt, mt * P:(mt + 1) * P],
                                rhs[rp][:, kt, csl],
                                start=(idx == 0), stop=(idx == n - 1))
                            idx += 1
                yield nch, mt, ps_r, ps_i

    def split_into(pool_or_tiles, src_ap, mt, csl, gen, plane):
        """src fp32 AP [P,512] -> hi/lo bf16 planes at [:, mt, csl]."""
        cur = pool_or_tiles
        hi = cur[plane + "h"][:, mt, csl]
        lo = cur[plane + "l"][:, mt, csl]
        nc.vector.tensor_copy(hi, src_ap)
        nc.vector.tensor_sub(lo, src_ap, hi)

    def new_cur(pool, gen):
        return {k: pool.tile([P, MT, D], BF16, tag=f"cur_{k}", bufs=1,
                             name=f"cur_{k}_{gen}")
                for k in ("rh", "rl", "ih", "il")}

    def ag_round(pool, cur, full_tag_gen):
        """DMA shard planes -> bounce, AllGather in group, DMA into full
        [P, KT, D] planes. Returns dict rh/rl/ih/il of full planes."""
        gen = full_tag_gen
        ag_in = pools["dram"].tile([4, SH, D], BF16, tag="ag_in", bufs=2,
                                   name=f"ag_in_{gen}")
        ag_out = pools["dram"].tile([GROUP, 4, SH, D], BF16, tag="ag_out",
                                    bufs=2, name=f"ag_out_{gen}")
        for pi, key in enumerate(("rh", "rl", "ih", "il")):
            nc.sync.dma_start(
                ag_in[pi].rearrange("(mt p) c -> p mt c", p=P), cur[key][:])
        nc.gpsimd.collective_compute(
            "AllGather", mybir.AluOpType.bypass, replica_groups=RG,
            ins=[ag_in[:].opt()], outs=[ag_out[:].opt()])
        full = {}
        for pi, key in enumerate(("rh", "rl", "ih", "il")):
            t = pools["core"].tile([P, KT, D], BF16, tag=f"full_{key}", bufs=1,
                                   name=f"full_{key}_{gen}")
            for r in range(GROUP):
                nc.sync.dma_start(
                    t[:, r * MT:(r + 1) * MT, :],
                    ag_out[r, pi].rearrange("(mt p) c -> p mt c", p=P))
            full[key] = t
        return full

    # ---------------- phase A: setup ------------------------------------
    expm_scope = ExitStack()
    core_pool = expm_scope.enter_context(tc.tile_pool(name="core", bufs=1))
    pools["core"] = core_pool
    with tc.tile_pool(name="taylor", bufs=1) as tay_pool, \
         tc.tile_pool(name="setup", bufs=1) as set_pool:
        # fp32 masters for the Taylor group sums (live until Horner ends)
        a0s = {}
        a2s = {}
        eye = tay_pool.tile([P, MT, D], F32, tag="eye")
        nc.sync.dma_start(eye[:], t_in["eye"].ap().rearrange(
            "(mt p) c -> p mt c", p=P))

        # a0 row shard fp32: 2^-6 * (rows of S)
        for plane, pos, neg in (("r", "arr", "artr"), ("i", "air", "aitr")):
            t1 = set_pool.tile([P, MT, D], F32, tag="t1", name=f"t1_{plane}")
            t2 = set_pool.tile([P, MT, D], F32, tag="t2", name=f"t2_{plane}")
            nc.sync.dma_start(t1[:], t_in[pos].ap().rearrange(
                "(mt p) c -> p mt c", p=P))
            nc.sync.dma_start(t2[:], t_in[neg].ap().rearrange(
                "(mt p) c -> p mt c", p=P))
            m = tay_pool.tile([P, MT, D], F32, tag=f"a0s_{plane}", name=f"a0s_{plane}")
            if plane == "r":
                nc.vector.tensor_sub(m[:], t1[:], t2[:])
            else:
                nc.vector.tensor_add(m[:], t1[:], t2[:])
            nc.vector.tensor_scalar_mul(m[:], m[:], SCALE)
            a0s[plane] = m

        # a0 full planes: 2^-6 * S, split to bf16 hi/lo, [P, KT, D]
        a0_full = {}
        for key in ("rh", "rl", "ih", "il"):
            a0_full[key] = core_pool.tile([P, KT, D], BF16, tag=f"full_{key}",
                                          bufs=1, name=f"full_{key}_a0")
        for plane, pos, neg in (("r", "ar", "art"), ("i", "ai", "ait")):
            for half in range(4):   # process 2 k-tiles at a time
                ksl = slice(half * 2, half * 2 + 2)
                rsl = slice(half * 256, half * 256 + 256)
                t1 = set_pool.tile([P, 2, D], F32, tag="t1",
                                   name=f"f1_{plane}_{half}")
                t2 = set_pool.tile([P, 2, D], F32, tag="t2",
                                   name=f"f2_{plane}_{half}")
                nc.sync.dma_start(t1[:], t_in[pos].ap()[rsl, :].rearrange(
                    "(kt p) c -> p kt c", p=P))
                nc.sync.dma_start(t2[:], t_in[neg].ap()[rsl, :].rearrange(
                    "(kt p) c -> p kt c", p=P))
                s32 = set_pool.tile([P, 2, D], F32, tag="s32",
                                    name=f"s32_{plane}_{half}")
                if plane == "r":
                    nc.vector.tensor_sub(s32[:], t1[:], t2[:])
                else:
                    nc.vector.tensor_add(s32[:], t1[:], t2[:])
                nc.vector.tensor_scalar_mul(s32[:], s32[:], SCALE)
                hi = a0_full[plane + "h"][:, ksl, :]
                nc.vector.tensor_copy(hi, s32[:])
                nc.vector.tensor_sub(a0_full[plane + "l"][:, ksl, :], s32[:], hi)

        # cur planes of A0 shard + first lhsT
        cur = new_cur(core_pool, "a0")
        for plane in ("r", "i"):
            for mt in range(MT):
                for nch in range(2):
                    csl = slice(nch * 512, (nch + 1) * 512)
                    split_into(cur, a0s[plane][:, mt, csl], mt, csl, "a0", plane)
        lhsT = make_lhsT(core_pool, cur, "a0")

        # ---------------- phase B: Taylor -------------------------------
        # A2 = A0s @ A0 (keep fp32 master), A3 = A2s @ A0 (AG -> full)
        for plane in ("r", "i"):
            a2s[plane] = tay_pool.tile([P, MT, D], F32, tag=f"a2s_{plane}", name=f"a2s_{plane}")
        cur = new_cur(core_pool, "a2")
        for nch, mt, ps_r, ps_i in emit_cmm_banks(lhsT, a0_full, "a2"):
            csl = slice(nch * 512, (nch + 1) * 512)
            for plane, bank in (("r", ps_r), ("i", ps_i)):
                nc.scalar.copy(a2s[plane][:, mt, csl], bank[:])
                split_into(cur, bank[:], mt, csl, "a2", plane)
        lhsT = make_lhsT(core_pool, cur, "a2")

        cur = new_cur(core_pool, "a3")
        for nch, mt, ps_r, ps_i in emit_cmm_banks(lhsT, a0_full, "a3"):
            csl = slice(nch * 512, (nch + 1) * 512)
            for plane, bank in (("r", ps_r), ("i", ps_i)):
                split_into(cur, bank[:], mt, csl, "a3", plane)
        a3_full = ag_round(core_pool, cur, "a3")   # reuses full_* slots

        # G_j builders (fp32 scratch [P, MT, D])
        def build_g(j, gen):
            c0, c1, c2 = CE[3 * j], CE[3 * j + 1], CE[3 * j + 2]
            gr = set_pool.tile([P, MT, D], F32, tag="t1", name=f"gr_{gen}")
            gi = set_pool.tile([P, MT, D], F32, tag="t2", name=f"gi_{gen}")
            nc.vector.tensor_scalar_mul(gr[:], eye[:], c0)
            nc.vector.scalar_tensor_tensor(
                gr[:], a0s["r"][:], c1, gr[:],
                mybir.AluOpType.mult, mybir.AluOpType.add)
            nc.vector.scalar_tensor_tensor(
                gr[:], a2s["r"][:], c2, gr[:],
                mybir.AluOpType.mult, mybir.AluOpType.add)
            nc.vector.tensor_scalar_mul(gi[:], a0s["i"][:], c1)
            nc.vector.scalar_tensor_tensor(
                gi[:], a2s["i"][:], c2, gi[:],
                mybir.AluOpType.mult, mybir.AluOpType.add)
            return {"r": gr, "i": gi}

        # T0 = G4 = c12 I + c13 A0 + c14 A2
        g4 = build_g(4, "g4")
        cur = new_cur(core_pool, "t0")
        for plane in ("r", "i"):
            for mt in range(MT):
                for nch in range(2):
                    csl = slice(nch * 512, (nch + 1) * 512)
                    split_into(cur, g4[plane][:, mt, csl], mt, csl, "t0", plane)
        lhsT = make_lhsT(core_pool, cur, "t0")

        # Horner: T_{j} = T_{j+1} @ A3 + G_j  for j = 3, 2, 1, 0
        for j in (3, 2, 1, 0):
            g = build_g(j, f"g{j}")
            cur = new_cur(core_pool, f"h{j}")
            s32s = []
            for nch, mt, ps_r, ps_i in emit_cmm_banks(lhsT, a3_full, f"h{j}"):
                csl = slice(nch * 512, (nch + 1) * 512)
                for plane, bank in (("r", ps_r), ("i", ps_i)):
                    s32 = io_pool.tile([P, 512], F32, tag="s32", bufs=2,
                                       name=f"s32_h{j}_{nch}_{mt}_{plane}")
                    nc.vector.tensor_add(s32[:], bank[:], g[plane][:, mt, csl])
                    split_into(cur, s32[:], mt, csl, f"h{j}", plane)
            lhsT = make_lhsT(core_pool, cur, f"h{j}")
        # taylor + setup pools die here

    # ---------------- phase C: squarings --------------------------------
    uag_out = pools["dram"].tile([GROUP, 2, D, SH], F32, tag="uag_out")
    with tc.tile_pool(name="usq", bufs=1) as usq_pool:
        u32 = None
        for sq in range(S_SQ):
            full = ag_round(core_pool, cur, f"sq{sq}")
            last = sq == S_SQ - 1
            if not last:
                cur = new_cur(core_pool, f"sq{sq}")
                for nch, mt, ps_r, ps_i in emit_cmm_banks(lhsT, full, f"sq{sq}"):
                    csl = slice(nch * 512, (nch + 1) * 512)
                    for plane, bank in (("r", ps_r), ("i", ps_i)):
                        split_into(cur, bank[:], mt, csl, f"sq{sq}", plane)
                lhsT = make_lhsT(core_pool, cur, f"sq{sq}")
            else:
                u32 = {p: usq_pool.tile([P, MT, D], F32, tag=f"u32_{p}",
                                        name=f"u32_{p}")
                       for p in ("r", "i")}
                for nch, mt, ps_r, ps_i in emit_cmm_banks(lhsT, full, "u"):
                    csl = slice(nch * 512, (nch + 1) * 512)
                    nc.vector.tensor_copy(u32["r"][:, mt, csl], ps_r[:])
                    nc.vector.tensor_copy(u32["i"][:, mt, csl], ps_i[:])

        # transpose U shard (fp32) -> UT col-shard [P, KT, SH], AG full U^T
        uts = {p: usq_pool.tile([P, KT, SH], F32, tag=f"uts_{p}",
                                name=f"uts_{p}")
               for p in ("r", "i")}
        for p in ("r", "i"):
            for mt in range(MT):
                for ct in range(KT):
                    pt = pst_pool.tile([P, P], F32, tag="pst",
                                       name=f"ptu_{p}_{mt}_{ct}")
                    nc.tensor.transpose(pt[:],
                                        u32[p][:, mt, ct * P:(ct + 1) * P],
                                        idn32[:])
                    nc.vector.tensor_copy(uts[p][:, ct, mt * P:(mt + 1) * P],
                                          pt[:])

        uag_in = pools["dram"].tile([2, D, SH], F32, tag="uag_in")
        for pi, p in enumerate(("r", "i")):
            nc.sync.dma_start(
                uag_in[pi].rearrange("(kt p) m -> p kt m", p=P), uts[p][:])
        nc.gpsimd.collective_compute(
            "AllGather", mybir.AluOpType.bypass, replica_groups=RG,
            ins=[uag_in[:].opt()], outs=[uag_out[:].opt()])

    expm_scope.close()

    # ---------------- phase D: state matmul (fp32r) ---------------------
    with tc.tile_pool(name="state", bufs=1) as st_pool:
        ut = {}
        for pi, p in enumerate(("r", "i")):
            t = st_pool.tile([P, KT, D], F32R, tag=f"ut_{p}", name=f"ut_{p}")
            for r in range(GROUP):
                nc.sync.dma_start(
                    t[:, :, r * SH:(r + 1) * SH],
                    uag_out[r, pi].rearrange("(kt p) m -> p kt m", p=P).bitcast(F32R))
            ut[p] = t
        nut = st_pool.tile([P, KT, D], F32R, tag="ut_n")
        nc.vector.tensor_scalar_mul(nut[:], ut["i"][:], -1.0)

        for b1 in range(XB):
            for xch in range(XC // 512):
                xo = slice(xch * 512, (xch + 1) * 512)
                xc = {}
                for p, nm in (("r", "xr"), ("i", "xi")):
                    t = st_pool.tile([P, KT, 512], F32R, tag=f"xc_{p}", bufs=2,
                                     name=f"xc_{p}_{b1}_{xch}")
                    nc.sync.dma_start(t[:], t_in[nm].ap()[b1, :, xo].rearrange(
                        "(kt p) c -> p kt c", p=P).bitcast(F32R))
                    xc[p] = t
                for mt in range(KT):    # 8 m-tiles over output rows
                    msl = slice(mt * P, (mt + 1) * P)
                    ps_r = ps_pool.tile([P, 512], F32, tag="ps",
                                        name=f"ysr_{b1}_{xch}_{mt}")
                    ps_i = ps_pool.tile([P, 512], F32, tag="ps",
                                        name=f"ysi_{b1}_{xch}_{mt}")
                    for bank, passes in (
                            (ps_r, ((ut["r"], xc["r"]), (nut, xc["i"]))),
                            (ps_i, ((ut["r"], xc["i"]), (ut["i"], xc["r"])))):
                        idx = 0
                        for w, m in passes:
                            for kt in range(KT):
                                nc.tensor.matmul(
                                    bank[:],
                                    w[:, kt, msl],
                                    m[:, kt, :],
                                    start=(idx == 0), stop=(idx == 15))
                                idx += 1
                    for pi, bank in ((0, ps_r), (1, ps_i)):
                        ys = io_pool.tile([P, 512], F32, tag="ys", bufs=2,
                                          name=f"ys_{pi}_{b1}_{xch}_{mt}")
                        nc.vector.tensor_copy(ys[:], bank[:])
                        nc.sync.dma_start(
                            t_out.ap()[pi, b1, mt * P:(mt + 1) * P, xo], ys[:])


_CACHED = {}


def _build_program():
    if "nc" in _CACHED:
        return _CACHED["nc"]
    nc = bacc.Bacc("TRN2", target_bir_lowering=False, debug=False,
                   num_devices=NCORES)
    t_in = {}
    for nm, shp in (("xr", [XB, D, XC]), ("xi", [XB, D, XC]),
                    ("ar", [D, D]), ("ai", [D, D]),
                    ("art", [D, D]), ("ait", [D, D]),
                    ("arr", [SH, D]), ("air", [SH, D]),
                    ("artr", [SH, D]), ("aitr", [SH, D]),
                    ("eye", [SH, D])):
        t_in[nm] = nc.dram_tensor(nm, shp, F32, kind="ExternalInput")
    t_in["idn"] = nc.dram_tensor("idn", [P, P], BF16, kind="ExternalInput")
    t_in["idn32"] = nc.dram_tensor("idn32", [P, P], F32, kind="ExternalInput")
    t_out = nc.dram_tensor("y", [2, XB, D, XC], F32, kind="ExternalOutput")

    with tile.TileContext(nc) as tc:
        with tc.tile_pool(name="io", bufs=1) as io_pool, \
             tc.tile_pool(name="ps", bufs=4, space="PSUM") as ps_pool, \
             tc.tile_pool(name="pst", bufs=4, space="PSUM") as pst_pool, \
             tc.tile_pool(name="dram", bufs=1, space="DRAM") as dram_pool:
            pools = {"io": io_pool, "ps": ps_pool,
                     "pst": pst_pool, "dram": dram_pool}
            _emit(nc, tc, t_in, t_out, pools)
    nc.compile()
    _CACHED["nc"] = nc
    return nc


def _in_maps(x_real, x_imag, U_real, U_imag):
    x4r = np.ascontiguousarray(x_real.reshape(2, 2, D, 4 * XC))
    x4i = np.ascontiguousarray(x_imag.reshape(2, 2, D, 4 * XC))
    idn = np.eye(P, dtype=ml_dtypes.bfloat16)
    idn32 = np.eye(P, dtype=np.float32)
    eye_full = np.eye(D, dtype=np.float32)
    maps = []
    for c in range(NCORES):
        k, r = divmod(c, GROUP)
        rows = slice(r * SH, (r + 1) * SH)
        cols = slice(r * XC, (r + 1) * XC)
        ar = np.ascontiguousarray(U_real[k])
        ai = np.ascontiguousarray(U_imag[k])
        art = np.ascontiguousarray(U_real[k].T)
        ait = np.ascontiguousarray(U_imag[k].T)
        maps.append({
            "xr": np.ascontiguousarray(x4r[k][:, :, cols]),
            "xi": np.ascontiguousarray(x4i[k][:, :, cols]),
            "ar": ar, "ai": ai, "art": art, "ait": ait,
            "arr": np.ascontiguousarray(ar[rows]),
            "air": np.ascontiguousarray(ai[rows]),
            "artr": np.ascontiguousarray(art[rows]),
            "aitr": np.ascontiguousarray(ait[rows]),
            "eye": np.ascontiguousarray(eye_full[rows]),
            "idn": idn, "idn32": idn32,
        })
    return maps


def _ensure_device_backend():
    """bass2jax dispatches through the default jax backend; make sure it is
    the neuron/axon one with all 8 cores visible (a harness may have pinned
    jax to cpu for its own reference computation)."""
    import jax

    try:
        devs = jax.devices()
        if devs and devs[0].platform != "cpu" and len(devs) >= NCORES:
            return
    except Exception:
        pass
    for plat in ("axon", "neuron"):
        try:
            jax.config.update("jax_platforms", plat)
            try:
                jax.clear_backends()
            except Exception:
                try:
                    from jax.extend.backend import clear_backends
                    clear_backends()
                except Exception:
                    pass
            devs = jax.devices()
            if devs and len(devs) >= NCORES:
                return
        except Exception:
            continue
    raise RuntimeError("could not find a jax backend with 8 neuron cores")


def kernel(x_real, x_imag, U_real, U_imag, _want_trace=False):
    x_real = np.asarray(x_real, dtype=np.float32).reshape(-1)
    x_imag = np.asarray(x_imag, dtype=np.float32).reshape(-1)
    U_real = np.asarray(U_real, dtype=np.float32)
    U_imag = np.asarray(U_imag, dtype=np.float32)
    _ensure_device_backend()

    nc = _build_program()
    maps = _in_maps(x_real, x_imag, U_real, U_imag)
    res = run_bass_kernel_spmd(nc, maps, core_ids=list(range(NCORES)),
                               trace=_want_trace)
    outs = [res.results[c]["y"] for c in range(NCORES)]

    full = np.empty((2, 2, 2, D, 4, XC), dtype=np.float32)
    for c in range(NCORES):
        k, r = divmod(c, GROUP)
        full[:, k, :, :, r, :] = outs[c]
    y = full.reshape(2, 1 << 24, 1)
    if _want_trace:
        return y, res
    return y


# revision 10
# speedup vs baseline: 1.1807x; 1.1807x over previous
"""Controlled-unitary gate (24 wires) on 8 trn2 NeuronCores.

Math: the reference's wire-permutation is pure index bookkeeping that cancels
in the flat layout.  With x4 = x.reshape(2,2,1024,4096) (control bit, b1,
target bits, low bits), the whole op is

    y4[k, b1] = expm(S_k) @ x4[k, b1],      S_k = A_k - A_k^H  (skew-Hermitian)

Device algorithm (per core; cores 0-3 handle control 0, cores 4-7 control 1;
within a group each core owns a 256-row shard of the 1024x1024 matrices and a
1024-column shard of the state):

  expm via scaling-and-squaring: A0 = S/2^6, degree-14 Taylor in
  Paterson-Stockmeyer form (powers A2, A3; Horner over A3), then 6 squarings.
  All expm matmuls run as bf16 hi/lo split products (3 passes per real
  product) accumulating in fp32 PSUM -> ~1e-4 relative error.
  Row-sharded chain: lhsT = transpose of the local shard (PE transpose),
  rhs = full matrix (replicated via AllGather inside each 4-core group).
  The final state multiply uses fp32r (FP22) matmuls at full PE rate.
"""

import os
from contextlib import ExitStack
from math import factorial

import numpy as np
import ml_dtypes

import concourse.bass as bass
import concourse.tile as tile
import concourse.mybir as mybir
from concourse import bacc
from concourse.bass_utils import run_bass_kernel_spmd

F32 = mybir.dt.float32
F32R = mybir.dt.float32r
BF16 = mybir.dt.bfloat16

D = 1024            # target-space dim
P = 128             # partitions
KT = D // P         # 8 k-tiles
SH = D // 4         # 256 rows per core shard
MT = SH // P        # 2 m-tiles per shard
S_SQ = 6            # squarings
M_TAY = 14          # Taylor degree (Paterson-Stockmeyer p=3)
CE = [1.0 / factorial(k) for k in range(M_TAY + 1)]
SCALE = 1.0 / (1 << S_SQ)
XB = 2              # b1 values
XC = 1024           # state columns per core per b1
NCORES = 8
GROUP = 4
RG = [[0, 1, 2, 3], [4, 5, 6, 7]]

# split-product pass lists: out_r += Zr*Rr - Zi*Ri ; out_i += Zr*Ri + Zi*Rr
# lhsT keys: rh/rl (Zr^T hi/lo), ih/il (Zi^T), nih/nil (-Zi^T)
_PASSES_R = [("rh", "rh"), ("rh", "rl"), ("rl", "rh"),
             ("nih", "ih"), ("nih", "il"), ("nil", "ih")]
_PASSES_I = [("rh", "ih"), ("rh", "il"), ("rl", "ih"),
             ("ih", "rh"), ("ih", "rl"), ("il", "rh")]


def _emit(nc, tc, t_in, t_out, pools):
    """Emit the whole per-core program (rank-oblivious).

    All 1024-wide matrices are kept as column HALVES (two tiles of 512 cols)
    so each AllGather round splits into two collectives that pipeline against
    the matmul chunks: while PE computes output columns 512-1023, the gather
    of columns 0-511 is already in flight.
    """
    ps_pool, pst_pool, io_pool = pools["ps"], pools["pst"], pools["io"]
    HALF = 512

    idn = io_pool.tile([P, P], BF16, tag="idn")
    idn32 = io_pool.tile([P, P], F32, tag="idn32")
    nc.sync.dma_start(idn[:], t_in["idn"].ap())
    nc.sync.dma_start(idn32[:], t_in["idn32"].ap())

    PLANES = ("rh", "rl", "ih", "il")

    # ---------------- helpers -------------------------------------------
    def new_cur(gen):
        return {k: [pools["core"].tile([P, MT, HALF], BF16, tag=f"cur_{k}{h}",
                                       bufs=1, name=f"cur_{k}{h}_{gen}")
                    for h in (0, 1)]
                for k in PLANES}

    def split_into(cur, src_ap, mt, nch, plane):
        hi = cur[plane + "h"][nch][:, mt, :]
        lo = cur[plane + "l"][nch][:, mt, :]
        nc.vector.tensor_copy(hi, src_ap)
        nc.vector.tensor_sub(lo, src_ap, hi)

    def make_lhsT(cur, gen, halves=(0, 1)):
        """PE-transpose cur shard planes into lhsT [P, KT, SH] planes."""
        if gen not in make_lhsT.__dict__.setdefault("gens", {}):
            make_lhsT.gens[gen] = {
                k: pools["core"].tile([P, KT, SH], BF16, tag=f"lt_{k}", bufs=2,
                                      name=f"lt_{k}_{gen}")
                for k in (*PLANES, "nih", "nil")}
        lt = make_lhsT.gens[gen]
        for h in halves:
            for key in PLANES:
                for mt in range(MT):
                    for c in range(4):
                        ct = h * 4 + c
                        pt = pst_pool.tile([P, P], BF16, tag="pst", bufs=2,
                                           name=f"pt_{gen}_{key}_{mt}_{ct}")
                        nc.tensor.transpose(
                            pt[:], cur[key][h][:, mt, c * P:(c + 1) * P],
                            idn[:])
                        nc.vector.tensor_copy(
                            lt[key][:, ct, mt * P:(mt + 1) * P], pt[:])
            for src_k, dst_k in (("ih", "nih"), ("il", "nil")):
                nc.vector.tensor_scalar_mul(
                    lt[dst_k][:, h * 4:(h + 1) * 4, :],
                    lt[src_k][:, h * 4:(h + 1) * 4, :], -1.0)
        return lt

    def emit_gather_half(cur, gen, h, full):
        """Bounce-DMA + AllGather of column-half h of the shard planes into
        full[key][h] tiles (replicated across the 4-core group)."""
        ag_in = pools["dram"].tile([4, SH, HALF], BF16, tag=f"ag_in{h}",
                                   bufs=2, name=f"ag_in{h}_{gen}")
        ag_out = pools["dram"].tile([GROUP, 4, SH, HALF], BF16,
                                    tag=f"ag_out{h}", bufs=2,
                                    name=f"ag_out{h}_{gen}")
        for pi, key in enumerate(PLANES):
            nc.sync.dma_start(
                ag_in[pi].rearrange("(mt p) c -> p mt c", p=P),
                cur[key][h][:])
        nc.gpsimd.collective_compute(
            "AllGather", mybir.AluOpType.bypass, replica_groups=RG,
            ins=[ag_in[:].opt()], outs=[ag_out[:].opt()])
        for pi, key in enumerate(PLANES):
            t = pools["core"].tile([P, KT, HALF], BF16, tag=f"full_{key}{h}",
                                   bufs=1, name=f"full_{key}{h}_{gen}")
            for r in range(GROUP):
                nc.sync.dma_start(
                    t[:, r * MT:(r + 1) * MT, :],
                    ag_out[r, pi].rearrange("(mt p) c -> p mt c", p=P))
            full[key] = full.get(key, [None, None])
            full[key][h] = t
        return full

    def emit_bank(bank, lhsT, rhs_half, passes, mt):
        n = len(passes) * KT
        idx = 0
        for lp, rp in passes:
            for kt in range(KT):
                nc.tensor.matmul(
                    bank[:],
                    lhsT[lp][:, kt, mt * P:(mt + 1) * P],
                    rhs_half[rp][:, kt, :],
                    start=(idx == 0), stop=(idx == n - 1))
                idx += 1

    def mm_step(lhsT, rhs, gen, evac, gather=False, transpose=True):
        """One shard@full complex matmul step.  rhs: key -> [half0, half1]
        tiles.  evac(nch, mt, ps_r, ps_i) must write `cur` planes.  If
        gather, each output half is AllGathered right after its evacs
        (pipelining with the other half's matmuls).  Returns gathered
        full-halves dict (or None)."""
        full = {} if gather else None
        for nch in (0, 1):
            rhs_half = {k: rhs[k][nch] for k in PLANES}
            for mt in range(MT):
                ps_r = ps_pool.tile([P, HALF], F32, tag="ps",
                                    name=f"psr_{gen}_{nch}_{mt}")
                ps_i = ps_pool.tile([P, HALF], F32, tag="ps",
                                    name=f"psi_{gen}_{nch}_{mt}")
                emit_bank(ps_r, lhsT, rhs_half, _PASSES_R, mt)
                emit_bank(ps_i, lhsT, rhs_half, _PASSES_I, mt)
                evac(nch, mt, ps_r, ps_i)
            if gather:
                emit_gather_half(mm_step.cur, gen, nch, full)
            if transpose:
                make_lhsT(mm_step.cur, gen, halves=(nch,))
        return full

    core_pool = None  # assigned below

    # ---------------- phase A: setup ------------------------------------
    expm_scope = ExitStack()
    core_pool = expm_scope.enter_context(tc.tile_pool(name="core", bufs=1))
    pools["core"] = core_pool
    with tc.tile_pool(name="taylor", bufs=1) as tay_pool, \
         tc.tile_pool(name="setup", bufs=1) as set_pool:
        # fp32 masters for the Taylor group sums (live until Horner ends)
        a0s = {}
        a2s = {}
        eye = tay_pool.tile([P, MT, D], F32, tag="eye")
        nc.sync.dma_start(eye[:], t_in["eye"].ap().rearrange(
            "(mt p) c -> p mt c", p=P))

        # a0 row shard fp32: 2^-6 * (rows of S)
        for plane, pos, neg in (("r", "arr", "artr"), ("i", "air", "aitr")):
            t1 = set_pool.tile([P, MT, D], F32, tag="t1", name=f"t1_{plane}")
            t2 = set_pool.tile([P, MT, D], F32, tag="t2", name=f"t2_{plane}")
            nc.sync.dma_start(t1[:], t_in[pos].ap().rearrange(
                "(mt p) c -> p mt c", p=P))
            nc.sync.dma_start(t2[:], t_in[neg].ap().rearrange(
                "(mt p) c -> p mt c", p=P))
            m = tay_pool.tile([P, MT, D], F32, tag=f"a0s_{plane}",
                              name=f"a0s_{plane}")
            if plane == "r":
                nc.vector.tensor_sub(m[:], t1[:], t2[:])
            else:
                nc.vector.tensor_add(m[:], t1[:], t2[:])
            nc.vector.tensor_scalar_mul(m[:], m[:], SCALE)
            a0s[plane] = m

        # a0 full planes (column halves): 2^-6 * S, split to bf16 hi/lo
        a0_full = {k: [core_pool.tile([P, KT, HALF], BF16, tag=f"full_{k}{h}",
                                      bufs=1, name=f"full_{k}{h}_a0")
                       for h in (0, 1)]
                   for k in PLANES}
        for plane, pos, neg in (("r", "ar", "art"), ("i", "ai", "ait")):
            for q in range(4):   # 2 k-tiles at a time
                ksl = slice(q * 2, q * 2 + 2)
                rsl = slice(q * 256, q * 256 + 256)
                t1 = set_pool.tile([P, 2, D], F32, tag="t1",
                                   name=f"f1_{plane}_{q}")
                t2 = set_pool.tile([P, 2, D], F32, tag="t2",
                                   name=f"f2_{plane}_{q}")
                nc.sync.dma_start(t1[:], t_in[pos].ap()[rsl, :].rearrange(
                    "(kt p) c -> p kt c", p=P))
                nc.sync.dma_start(t2[:], t_in[neg].ap()[rsl, :].rearrange(
                    "(kt p) c -> p kt c", p=P))
                s32 = set_pool.tile([P, 2, D], F32, tag="s32",
                                    name=f"s32_{plane}_{q}")
                if plane == "r":
                    nc.vector.tensor_sub(s32[:], t1[:], t2[:])
                else:
                    nc.vector.tensor_add(s32[:], t1[:], t2[:])
                nc.vector.tensor_scalar_mul(s32[:], s32[:], SCALE)
                for h in (0, 1):
                    hsl = slice(h * HALF, (h + 1) * HALF)
                    hi = a0_full[plane + "h"][h][:, ksl, :]
                    nc.vector.tensor_copy(hi, s32[:, :, hsl])
                    nc.vector.tensor_sub(a0_full[plane + "l"][h][:, ksl, :],
                                         s32[:, :, hsl], hi)

        # cur planes of A0 shard + first lhsT
        cur = new_cur("a0")
        for plane in ("r", "i"):
            for mt in range(MT):
                for nch in range(2):
                    csl = slice(nch * HALF, (nch + 1) * HALF)
                    split_into(cur, a0s[plane][:, mt, csl], mt, nch, plane)
        mm_step.cur = cur
        lhsT = make_lhsT(cur, "a0")

        # ---------------- phase B: Taylor -------------------------------
        for plane in ("r", "i"):
            a2s[plane] = tay_pool.tile([P, MT, D], F32, tag=f"a2s_{plane}",
                                       name=f"a2s_{plane}")

        def evac_a2(nch, mt, ps_r, ps_i):
            csl = slice(nch * HALF, (nch + 1) * HALF)
            for plane, bank in (("r", ps_r), ("i", ps_i)):
                nc.scalar.copy(a2s[plane][:, mt, csl], bank[:])
                split_into(mm_step.cur, bank[:], mt, nch, plane)

        mm_step.cur = new_cur("a2")
        mm_step(lhsT, a0_full, "a2", evac_a2)
        lhsT = make_lhsT.gens["a2"]

        def evac_plain(nch, mt, ps_r, ps_i):
            for plane, bank in (("r", ps_r), ("i", ps_i)):
                split_into(mm_step.cur, bank[:], mt, nch, plane)

        mm_step.cur = new_cur("a3")
        a3_full = mm_step(lhsT, a0_full, "a3", evac_plain, gather=True,
                          transpose=False)

        # G_j builders (fp32 scratch [P, MT, D])
        def build_g(j, gen):
            c0, c1, c2 = CE[3 * j], CE[3 * j + 1], CE[3 * j + 2]
            gr = set_pool.tile([P, MT, D], F32, tag="t1", name=f"gr_{gen}")
            gi = set_pool.tile([P, MT, D], F32, tag="t2", name=f"gi_{gen}")
            nc.vector.tensor_scalar_mul(gr[:], eye[:], c0)
            nc.vector.scalar_tensor_tensor(
                gr[:], a0s["r"][:], c1, gr[:],
                mybir.AluOpType.mult, mybir.AluOpType.add)
            nc.vector.scalar_tensor_tensor(
                gr[:], a2s["r"][:], c2, gr[:],
                mybir.AluOpType.mult, mybir.AluOpType.add)
            nc.vector.tensor_scalar_mul(gi[:], a0s["i"][:], c1)
            nc.vector.scalar_tensor_tensor(
                gi[:], a2s["i"][:], c2, gi[:],
                mybir.AluOpType.mult, mybir.AluOpType.add)
            return {"r": gr, "i": gi}

        # T0 = G4 = c12 I + c13 A0 + c14 A2
        g4 = build_g(4, "g4")
        cur = new_cur("t0")
        for plane in ("r", "i"):
            for mt in range(MT):
                for nch in range(2):
                    csl = slice(nch * HALF, (nch + 1) * HALF)
                    split_into(cur, g4[plane][:, mt, csl], mt, nch, plane)
        mm_step.cur = cur
        lhsT = make_lhsT(cur, "t0")

        # Horner: T_{j} = T_{j+1} @ A3 + G_j  for j = 3, 2, 1, 0
        for j in (3, 2, 1, 0):
            g = build_g(j, f"g{j}")

            def evac_g(nch, mt, ps_r, ps_i, g=g, j=j):
                csl = slice(nch * HALF, (nch + 1) * HALF)
                for plane, bank in (("r", ps_r), ("i", ps_i)):
                    s32 = io_pool.tile([P, HALF], F32, tag="s32", bufs=2,
                                       name=f"s32_h{j}_{nch}_{mt}_{plane}")
                    nc.vector.tensor_add(s32[:], bank[:],
                                         g[plane][:, mt, csl])
                    split_into(mm_step.cur, s32[:], mt, nch, plane)

            mm_step.cur = new_cur(f"h{j}")
            res = mm_step(lhsT, a3_full, f"h{j}", evac_g,
                          gather=(j == 0), transpose=(j != 0))
            if j != 0:
                lhsT = make_lhsT.gens[f"h{j}"]
            else:
                h0_full = res
    # taylor + setup pools die here

    # ---------------- phase C: squarings --------------------------------
    uag_out = [pools["dram"].tile([GROUP, 2, HALF, SH], F32,
                                  tag=f"uag_out{h}", name=f"uag_out{h}")
               for h in (0, 1)]
    with tc.tile_pool(name="usq", bufs=1) as usq_pool:
        # the j==0 Horner step above gathered its output but did not build
        # lhsT; do that now from its cur planes
        full = h0_full
        lhsT = make_lhsT(mm_step.cur, "sq_in")
        u32 = None
        for sq in range(S_SQ):
            last = sq == S_SQ - 1
            if not last:
                def evac_sq(nch, mt, ps_r, ps_i):
                    for plane, bank in (("r", ps_r), ("i", ps_i)):
                        split_into(mm_step.cur, bank[:], mt, nch, plane)

                mm_step.cur = new_cur(f"sq{sq}")
                full = mm_step(lhsT, full, f"sq{sq}", evac_sq, gather=True,
                               transpose=True)
                lhsT = make_lhsT.gens[f"sq{sq}"]
            else:
                u32 = {p: usq_pool.tile([P, MT, D], F32, tag=f"u32_{p}",
                                        name=f"u32_{p}")
                       for p in ("r", "i")}

                def evac_u(nch, mt, ps_r, ps_i):
                    csl = slice(nch * HALF, (nch + 1) * HALF)
                    nc.vector.tensor_copy(u32["r"][:, mt, csl], ps_r[:])
                    nc.vector.tensor_copy(u32["i"][:, mt, csl], ps_i[:])

                mm_step(lhsT, full, "u", evac_u, gather=False,
                        transpose=False)

        # transpose U shard (fp32) -> UT col-shard [P, KT, SH]; AG full U^T
        # in two k-halves so the state matmul can start on kt 0-3 early.
        uts = {p: usq_pool.tile([P, KT, SH], F32, tag=f"uts_{p}",
                                name=f"uts_{p}")
               for p in ("r", "i")}
        for p in ("r", "i"):
            for mt in range(MT):
                for ct in range(KT):
                    pt = pst_pool.tile([P, P], F32, tag="pst32", bufs=2,
                                       name=f"ptu_{p}_{mt}_{ct}")
                    nc.tensor.transpose(pt[:],
                                        u32[p][:, mt, ct * P:(ct + 1) * P],
                                        idn32[:])
                    nc.vector.tensor_copy(uts[p][:, ct, mt * P:(mt + 1) * P],
                                          pt[:])

        for h in (0, 1):
            uag_in = pools["dram"].tile([2, HALF, SH], F32, tag=f"uag_in{h}",
                                        name=f"uag_in{h}")
            for pi, p in enumerate(("r", "i")):
                nc.sync.dma_start(
                    uag_in[pi].rearrange("(kt p) m -> p kt m", p=P),
                    uts[p][:, h * 4:(h + 1) * 4, :])
            nc.gpsimd.collective_compute(
                "AllGather", mybir.AluOpType.bypass, replica_groups=RG,
                ins=[uag_in[:].opt()], outs=[uag_out[h][:].opt()])

    expm_scope.close()

    # ---------------- phase D: state matmul (fp32r) ---------------------
    with tc.tile_pool(name="state", bufs=1) as st_pool:
        ut = {}
        for pi, p in enumerate(("r", "i")):
            halves = []
            for h in (0, 1):
                t = st_pool.tile([P, 4, D], F32R, tag=f"ut_{p}{h}",
                                 name=f"ut_{p}{h}")
                for r in range(GROUP):
                    nc.sync.dma_start(
                        t[:, :, r * SH:(r + 1) * SH],
                        uag_out[h][r, pi].rearrange(
                            "(kt p) m -> p kt m", p=P).bitcast(F32R))
                halves.append(t)
            ut[p] = halves
        nut = []
        for h in (0, 1):
            t = st_pool.tile([P, 4, D], F32R, tag=f"ut_n{h}", name=f"nut{h}")
            nc.vector.tensor_scalar_mul(t[:], ut["i"][h][:], -1.0)
            nut.append(t)

        for b1 in range(XB):
            for xch in range(XC // 512):
                xo = slice(xch * 512, (xch + 1) * 512)
                xc = {}
                for p, nm in (("r", "xr"), ("i", "xi")):
                    t = st_pool.tile([P, KT, 512], F32R, tag=f"xc_{p}",
                                     bufs=2, name=f"xc_{p}_{b1}_{xch}")
                    nc.sync.dma_start(t[:], t_in[nm].ap()[b1, :, xo].rearrange(
                        "(kt p) c -> p kt c", p=P).bitcast(F32R))
                    xc[p] = t
                for mt in range(KT):    # 8 m-tiles over output rows
                    msl = slice(mt * P, (mt + 1) * P)
                    ps_r = ps_pool.tile([P, 512], F32, tag="ps",
                                        name=f"ysr_{b1}_{xch}_{mt}")
                    ps_i = ps_pool.tile([P, 512], F32, tag="ps",
                                        name=f"ysi_{b1}_{xch}_{mt}")
                    for bank, passes in (
                            (ps_r, ((ut["r"], xc["r"]), (nut, xc["i"]))),
                            (ps_i, ((ut["r"], xc["i"]), (ut["i"], xc["r"])))):
                        idx = 0
                        for h in (0, 1):
                            for w, m in passes:
                                wt = w[h] if isinstance(w, list) else w[h]
                                for kt in range(4):
                                    nc.tensor.matmul(
                                        ps_r[:] if bank is ps_r else ps_i[:],
                                        wt[:, kt, msl],
                                        m[:, h * 4 + kt, :],
                                        start=(idx == 0), stop=(idx == 15))
                                    idx += 1
                    for pi, bank in ((0, ps_r), (1, ps_i)):
                        ys = io_pool.tile([P, 512], F32, tag="ys", bufs=2,
                                          name=f"ys_{pi}_{b1}_{xch}_{mt}")
                        nc.vector.tensor_copy(ys[:], bank[:])
                        nc.sync.dma_start(
                            t_out.ap()[pi, b1, mt * P:(mt + 1) * P, xo], ys[:])


_CACHED = {}


def _build_program():
    if "nc" in _CACHED:
        return _CACHED["nc"]
    nc = bacc.Bacc("TRN2", target_bir_lowering=False, debug=False,
                   num_devices=NCORES)
    t_in = {}
    for nm, shp in (("xr", [XB, D, XC]), ("xi", [XB, D, XC]),
                    ("ar", [D, D]), ("ai", [D, D]),
                    ("art", [D, D]), ("ait", [D, D]),
                    ("arr", [SH, D]), ("air", [SH, D]),
                    ("artr", [SH, D]), ("aitr", [SH, D]),
                    ("eye", [SH, D])):
        t_in[nm] = nc.dram_tensor(nm, shp, F32, kind="ExternalInput")
    t_in["idn"] = nc.dram_tensor("idn", [P, P], BF16, kind="ExternalInput")
    t_in["idn32"] = nc.dram_tensor("idn32", [P, P], F32, kind="ExternalInput")
    t_out = nc.dram_tensor("y", [2, XB, D, XC], F32, kind="ExternalOutput")

    with tile.TileContext(nc) as tc:
        with tc.tile_pool(name="io", bufs=1) as io_pool, \
             tc.tile_pool(name="ps", bufs=4, space="PSUM") as ps_pool, \
             tc.tile_pool(name="pst", bufs=4, space="PSUM") as pst_pool, \
             tc.tile_pool(name="dram", bufs=1, space="DRAM") as dram_pool:
            pools = {"io": io_pool, "ps": ps_pool,
                     "pst": pst_pool, "dram": dram_pool}
            _emit(nc, tc, t_in, t_out, pools)
    nc.compile()
    _CACHED["nc"] = nc
    return nc


def _in_maps(x_real, x_imag, U_real, U_imag):
    x4r = np.ascontiguousarray(x_real.reshape(2, 2, D, 4 * XC))
    x4i = np.ascontiguousarray(x_imag.reshape(2, 2, D, 4 * XC))
    idn = np.eye(P, dtype=ml_dtypes.bfloat16)
    idn32 = np.eye(P, dtype=np.float32)
    eye_full = np.eye(D, dtype=np.float32)
    maps = []
    for c in range(NCORES):
        k, r = divmod(c, GROUP)
        rows = slice(r * SH, (r + 1) * SH)
        cols = slice(r * XC, (r + 1) * XC)
        ar = np.ascontiguousarray(U_real[k])
        ai = np.ascontiguousarray(U_imag[k])
        art = np.ascontiguousarray(U_real[k].T)
        ait = np.ascontiguousarray(U_imag[k].T)
        maps.append({
            "xr": np.ascontiguousarray(x4r[k][:, :, cols]),
            "xi": np.ascontiguousarray(x4i[k][:, :, cols]),
            "ar": ar, "ai": ai, "art": art, "ait": ait,
            "arr": np.ascontiguousarray(ar[rows]),
            "air": np.ascontiguousarray(ai[rows]),
            "artr": np.ascontiguousarray(art[rows]),
            "aitr": np.ascontiguousarray(ait[rows]),
            "eye": np.ascontiguousarray(eye_full[rows]),
            "idn": idn, "idn32": idn32,
        })
    return maps


def _ensure_device_backend():
    """bass2jax dispatches through the default jax backend; make sure it is
    the neuron/axon one with all 8 cores visible (a harness may have pinned
    jax to cpu for its own reference computation)."""
    import jax

    try:
        devs = jax.devices()
        if devs and devs[0].platform != "cpu" and len(devs) >= NCORES:
            return
    except Exception:
        pass
    for plat in ("axon", "neuron"):
        try:
            jax.config.update("jax_platforms", plat)
            try:
                jax.clear_backends()
            except Exception:
                try:
                    from jax.extend.backend import clear_backends
                    clear_backends()
                except Exception:
                    pass
            devs = jax.devices()
            if devs and len(devs) >= NCORES:
                return
        except Exception:
            continue
    raise RuntimeError("could not find a jax backend with 8 neuron cores")


def kernel(x_real, x_imag, U_real, U_imag, _want_trace=False):
    x_real = np.asarray(x_real, dtype=np.float32).reshape(-1)
    x_imag = np.asarray(x_imag, dtype=np.float32).reshape(-1)
    U_real = np.asarray(U_real, dtype=np.float32)
    U_imag = np.asarray(U_imag, dtype=np.float32)
    _ensure_device_backend()

    nc = _build_program()
    maps = _in_maps(x_real, x_imag, U_real, U_imag)
    res = run_bass_kernel_spmd(nc, maps, core_ids=list(range(NCORES)),
                               trace=_want_trace)
    outs = [res.results[c]["y"] for c in range(NCORES)]

    full = np.empty((2, 2, 2, D, 4, XC), dtype=np.float32)
    for c in range(NCORES):
        k, r = divmod(c, GROUP)
        full[:, k, :, :, r, :] = outs[c]
    y = full.reshape(2, 1 << 24, 1)
    if _want_trace:
        return y, res
    return y


# revision 11
# speedup vs baseline: 1.2106x; 1.0253x over previous
"""Controlled-unitary gate (24 wires) on 8 trn2 NeuronCores.

Math: the reference's wire-permutation is pure index bookkeeping that cancels
in the flat layout.  With x4 = x.reshape(2,2,1024,4096) (control bit, b1,
target bits, low bits), the whole op is

    y4[k, b1] = expm(S_k) @ x4[k, b1],      S_k = A_k - A_k^H  (skew-Hermitian)

Device algorithm (per core; cores 0-3 handle control 0, cores 4-7 control 1;
within a group each core owns a 256-row shard of the 1024x1024 matrices and a
1024-column shard of the state):

  expm via scaling-and-squaring: A0 = S/2^6, degree-14 Taylor in
  Paterson-Stockmeyer form (powers A2, A3; Horner over A3), then 6 squarings.
  All expm matmuls run as bf16 hi/lo split products (3 passes per real
  product) accumulating in fp32 PSUM -> ~1e-4 relative error.
  Row-sharded chain: lhsT = transpose of the local shard (PE transpose),
  rhs = full matrix (replicated via AllGather inside each 4-core group).
  The final state multiply uses fp32r (FP22) matmuls at full PE rate.
"""

import os
from contextlib import ExitStack
from math import factorial

import numpy as np
import ml_dtypes

import concourse.bass as bass
import concourse.tile as tile
import concourse.mybir as mybir
from concourse import bacc
from concourse.bass_utils import run_bass_kernel_spmd

F32 = mybir.dt.float32
F32R = mybir.dt.float32r
BF16 = mybir.dt.bfloat16

D = 1024            # target-space dim
P = 128             # partitions
KT = D // P         # 8 k-tiles
SH = D // 4         # 256 rows per core shard
MT = SH // P        # 2 m-tiles per shard
S_SQ = 6            # squarings
M_TAY = 14          # Taylor degree (Paterson-Stockmeyer p=3)
CE = [1.0 / factorial(k) for k in range(M_TAY + 1)]
SCALE = 1.0 / (1 << S_SQ)
XB = 2              # b1 values
XC = 1024           # state columns per core per b1
NCORES = 8
GROUP = 4
RG = [[0, 1, 2, 3], [4, 5, 6, 7]]

# split-product pass lists: out_r += Zr*Rr - Zi*Ri ; out_i += Zr*Ri + Zi*Rr
# lhsT keys: rh/rl (Zr^T hi/lo), ih/il (Zi^T), nih/nil (-Zi^T)
_PASSES_R = [("rh", "rh"), ("rh", "rl"), ("rl", "rh"),
             ("nih", "ih"), ("nih", "il"), ("nil", "ih")]
_PASSES_I = [("rh", "ih"), ("rh", "il"), ("rl", "ih"),
             ("ih", "rh"), ("ih", "rl"), ("il", "rh")]


def _emit(nc, tc, t_in, t_out, pools):
    """Emit the whole per-core program (rank-oblivious).

    All 1024-wide matrices are kept as column HALVES (two tiles of 512 cols)
    so each AllGather round splits into two collectives that pipeline against
    the matmul chunks: while PE computes output columns 512-1023, the gather
    of columns 0-511 is already in flight.
    """
    ps_pool, pst_pool, io_pool = pools["ps"], pools["pst"], pools["io"]
    HALF = 512

    idn = io_pool.tile([P, P], BF16, tag="idn")
    idn32 = io_pool.tile([P, P], F32, tag="idn32")
    nc.sync.dma_start(idn[:], t_in["idn"].ap())
    nc.sync.dma_start(idn32[:], t_in["idn32"].ap())

    PLANES = ("rh", "rl", "ih", "il")

    # ---------------- helpers -------------------------------------------
    def new_cur(gen):
        return {k: [pools["core"].tile([P, MT, HALF], BF16, tag=f"cur_{k}{h}",
                                       bufs=1, name=f"cur_{k}{h}_{gen}")
                    for h in (0, 1)]
                for k in PLANES}

    def split_into(cur, src_ap, mt, nch, plane):
        hi = cur[plane + "h"][nch][:, mt, :]
        lo = cur[plane + "l"][nch][:, mt, :]
        nc.vector.tensor_copy(hi, src_ap)
        nc.vector.tensor_sub(lo, src_ap, hi)

    def make_lhsT(cur, gen, halves=(0, 1)):
        """PE-transpose cur shard planes into lhsT [P, KT, SH] planes."""
        if gen not in make_lhsT.__dict__.setdefault("gens", {}):
            make_lhsT.gens[gen] = {
                k: pools["core"].tile([P, KT, SH], BF16, tag=f"lt_{k}", bufs=2,
                                      name=f"lt_{k}_{gen}")
                for k in (*PLANES, "nih", "nil")}
        lt = make_lhsT.gens[gen]
        for h in halves:
            for key in PLANES:
                for mt in range(MT):
                    for c in range(4):
                        ct = h * 4 + c
                        pt = pst_pool.tile([P, P], BF16, tag="pst", bufs=2,
                                           name=f"pt_{gen}_{key}_{mt}_{ct}")
                        nc.tensor.transpose(
                            pt[:], cur[key][h][:, mt, c * P:(c + 1) * P],
                            idn[:])
                        nc.vector.tensor_copy(
                            lt[key][:, ct, mt * P:(mt + 1) * P], pt[:])
            for src_k, dst_k in (("ih", "nih"), ("il", "nil")):
                nc.vector.tensor_scalar_mul(
                    lt[dst_k][:, h * 4:(h + 1) * 4, :],
                    lt[src_k][:, h * 4:(h + 1) * 4, :], -1.0)
        return lt

    def emit_gather_half(cur, gen, h, full):
        """Bounce-DMA + AllGather of column-half h of the shard planes into
        full[key][h] tiles (replicated across the 4-core group)."""
        ag_in = pools["dram"].tile([4, SH, HALF], BF16, tag=f"ag_in{h}",
                                   bufs=2, name=f"ag_in{h}_{gen}")
        ag_out = pools["dram"].tile([GROUP, 4, SH, HALF], BF16,
                                    tag=f"ag_out{h}", bufs=2,
                                    name=f"ag_out{h}_{gen}")
        for pi, key in enumerate(PLANES):
            nc.sync.dma_start(
                ag_in[pi].rearrange("(mt p) c -> p mt c", p=P),
                cur[key][h][:])
        nc.gpsimd.collective_compute(
            "AllGather", mybir.AluOpType.bypass, replica_groups=RG,
            ins=[ag_in[:].opt()], outs=[ag_out[:].opt()])
        for pi, key in enumerate(PLANES):
            t = pools["core"].tile([P, KT, HALF], BF16, tag=f"full_{key}{h}",
                                   bufs=1, name=f"full_{key}{h}_{gen}")
            for r in range(GROUP):
                nc.scalar.dma_start(
                    t[:, r * MT:(r + 1) * MT, :],
                    ag_out[r, pi].rearrange("(mt p) c -> p mt c", p=P))
            full[key] = full.get(key, [None, None])
            full[key][h] = t
        return full

    def emit_bank(bank, lhsT, rhs_half, passes, mt):
        n = len(passes) * KT
        idx = 0
        for lp, rp in passes:
            for kt in range(KT):
                nc.tensor.matmul(
                    bank[:],
                    lhsT[lp][:, kt, mt * P:(mt + 1) * P],
                    rhs_half[rp][:, kt, :],
                    start=(idx == 0), stop=(idx == n - 1))
                idx += 1

    def mm_step(lhsT, rhs, gen, evac, gather=False, transpose=True,
                after_half=None):
        """One shard@full complex matmul step.  rhs: key -> [half0, half1]
        tiles.  evac(nch, mt, ps_r, ps_i) must write `cur` planes.  If
        gather, each output half is AllGathered right after its evacs
        (pipelining with the other half's matmuls).  Returns gathered
        full-halves dict (or None)."""
        full = {} if gather else None
        for nch in (0, 1):
            rhs_half = {k: rhs[k][nch] for k in PLANES}
            for mt in range(MT):
                ps_r = ps_pool.tile([P, HALF], F32, tag="ps",
                                    name=f"psr_{gen}_{nch}_{mt}")
                ps_i = ps_pool.tile([P, HALF], F32, tag="ps",
                                    name=f"psi_{gen}_{nch}_{mt}")
                emit_bank(ps_r, lhsT, rhs_half, _PASSES_R, mt)
                emit_bank(ps_i, lhsT, rhs_half, _PASSES_I, mt)
                evac(nch, mt, ps_r, ps_i)
            if gather:
                emit_gather_half(mm_step.cur, gen, nch, full)
            if transpose:
                make_lhsT(mm_step.cur, gen, halves=(nch,))
            if after_half is not None:
                after_half(nch)
        return full

    core_pool = None  # assigned below

    # ---------------- phase A: setup ------------------------------------
    expm_scope = ExitStack()
    core_pool = expm_scope.enter_context(tc.tile_pool(name="core", bufs=1))
    pools["core"] = core_pool
    with tc.tile_pool(name="taylor", bufs=1) as tay_pool, \
         tc.tile_pool(name="setup", bufs=1) as set_pool:
        # fp32 masters for the Taylor group sums (live until Horner ends)
        a0s = {}
        a2s = {}
        eye = tay_pool.tile([P, MT, D], F32, tag="eye")
        nc.sync.dma_start(eye[:], t_in["eye"].ap().rearrange(
            "(mt p) c -> p mt c", p=P))

        # a0 row shard fp32: 2^-6 * (rows of S)
        for plane, pos, neg in (("r", "arr", "artr"), ("i", "air", "aitr")):
            t1 = set_pool.tile([P, MT, D], F32, tag="t1", name=f"t1_{plane}")
            t2 = set_pool.tile([P, MT, D], F32, tag="t2", name=f"t2_{plane}")
            nc.sync.dma_start(t1[:], t_in[pos].ap().rearrange(
                "(mt p) c -> p mt c", p=P))
            nc.sync.dma_start(t2[:], t_in[neg].ap().rearrange(
                "(mt p) c -> p mt c", p=P))
            m = tay_pool.tile([P, MT, D], F32, tag=f"a0s_{plane}",
                              name=f"a0s_{plane}")
            if plane == "r":
                nc.vector.tensor_sub(m[:], t1[:], t2[:])
            else:
                nc.vector.tensor_add(m[:], t1[:], t2[:])
            nc.vector.tensor_scalar_mul(m[:], m[:], SCALE)
            a0s[plane] = m

        # a0 full planes (column halves): 2^-6 * S, split to bf16 hi/lo
        a0_full = {k: [core_pool.tile([P, KT, HALF], BF16, tag=f"full_{k}{h}",
                                      bufs=1, name=f"full_{k}{h}_a0")
                       for h in (0, 1)]
                   for k in PLANES}
        for plane, pos, neg in (("r", "ar", "art"), ("i", "ai", "ait")):
            for q in range(4):   # 2 k-tiles at a time
                ksl = slice(q * 2, q * 2 + 2)
                rsl = slice(q * 256, q * 256 + 256)
                t1 = set_pool.tile([P, 2, D], F32, tag="t1",
                                   name=f"f1_{plane}_{q}")
                t2 = set_pool.tile([P, 2, D], F32, tag="t2",
                                   name=f"f2_{plane}_{q}")
                nc.sync.dma_start(t1[:], t_in[pos].ap()[rsl, :].rearrange(
                    "(kt p) c -> p kt c", p=P))
                nc.sync.dma_start(t2[:], t_in[neg].ap()[rsl, :].rearrange(
                    "(kt p) c -> p kt c", p=P))
                s32 = set_pool.tile([P, 2, D], F32, tag="s32",
                                    name=f"s32_{plane}_{q}")
                if plane == "r":
                    nc.vector.tensor_sub(s32[:], t1[:], t2[:])
                else:
                    nc.vector.tensor_add(s32[:], t1[:], t2[:])
                nc.vector.tensor_scalar_mul(s32[:], s32[:], SCALE)
                for h in (0, 1):
                    hsl = slice(h * HALF, (h + 1) * HALF)
                    hi = a0_full[plane + "h"][h][:, ksl, :]
                    nc.vector.tensor_copy(hi, s32[:, :, hsl])
                    nc.vector.tensor_sub(a0_full[plane + "l"][h][:, ksl, :],
                                         s32[:, :, hsl], hi)

        # cur planes of A0 shard + first lhsT
        cur = new_cur("a0")
        for plane in ("r", "i"):
            for mt in range(MT):
                for nch in range(2):
                    csl = slice(nch * HALF, (nch + 1) * HALF)
                    split_into(cur, a0s[plane][:, mt, csl], mt, nch, plane)
        mm_step.cur = cur
        lhsT = make_lhsT(cur, "a0")

        # ---------------- phase B: Taylor -------------------------------
        for plane in ("r", "i"):
            a2s[plane] = tay_pool.tile([P, MT, D], F32, tag=f"a2s_{plane}",
                                       name=f"a2s_{plane}")

        def evac_a2(nch, mt, ps_r, ps_i):
            csl = slice(nch * HALF, (nch + 1) * HALF)
            for plane, bank in (("r", ps_r), ("i", ps_i)):
                nc.scalar.copy(a2s[plane][:, mt, csl], bank[:])
                split_into(mm_step.cur, bank[:], mt, nch, plane)

        mm_step.cur = new_cur("a2")
        mm_step(lhsT, a0_full, "a2", evac_a2)
        lhsT = make_lhsT.gens["a2"]

        def evac_plain(nch, mt, ps_r, ps_i):
            for plane, bank in (("r", ps_r), ("i", ps_i)):
                split_into(mm_step.cur, bank[:], mt, nch, plane)

        mm_step.cur = new_cur("a3")
        a3_full = mm_step(lhsT, a0_full, "a3", evac_plain, gather=True,
                          transpose=False)

        # G_j builders (fp32 scratch [P, MT, D])
        def build_g(j, gen):
            c0, c1, c2 = CE[3 * j], CE[3 * j + 1], CE[3 * j + 2]
            gr = set_pool.tile([P, MT, D], F32, tag="t1", name=f"gr_{gen}")
            gi = set_pool.tile([P, MT, D], F32, tag="t2", name=f"gi_{gen}")
            nc.vector.tensor_scalar_mul(gr[:], eye[:], c0)
            nc.vector.scalar_tensor_tensor(
                gr[:], a0s["r"][:], c1, gr[:],
                mybir.AluOpType.mult, mybir.AluOpType.add)
            nc.vector.scalar_tensor_tensor(
                gr[:], a2s["r"][:], c2, gr[:],
                mybir.AluOpType.mult, mybir.AluOpType.add)
            nc.vector.tensor_scalar_mul(gi[:], a0s["i"][:], c1)
            nc.vector.scalar_tensor_tensor(
                gi[:], a2s["i"][:], c2, gi[:],
                mybir.AluOpType.mult, mybir.AluOpType.add)
            return {"r": gr, "i": gi}

        # T0 = G4 = c12 I + c13 A0 + c14 A2
        g4 = build_g(4, "g4")
        cur = new_cur("t0")
        for plane in ("r", "i"):
            for mt in range(MT):
                for nch in range(2):
                    csl = slice(nch * HALF, (nch + 1) * HALF)
                    split_into(cur, g4[plane][:, mt, csl], mt, nch, plane)
        mm_step.cur = cur
        lhsT = make_lhsT(cur, "t0")

        # Horner: T_{j} = T_{j+1} @ A3 + G_j  for j = 3, 2, 1, 0
        for j in (3, 2, 1, 0):
            g = build_g(j, f"g{j}")

            def evac_g(nch, mt, ps_r, ps_i, g=g, j=j):
                csl = slice(nch * HALF, (nch + 1) * HALF)
                for plane, bank in (("r", ps_r), ("i", ps_i)):
                    s32 = io_pool.tile([P, HALF], F32, tag="s32", bufs=2,
                                       name=f"s32_h{j}_{nch}_{mt}_{plane}")
                    nc.vector.tensor_add(s32[:], bank[:],
                                         g[plane][:, mt, csl])
                    split_into(mm_step.cur, s32[:], mt, nch, plane)

            mm_step.cur = new_cur(f"h{j}")
            res = mm_step(lhsT, a3_full, f"h{j}", evac_g,
                          gather=(j == 0), transpose=(j != 0))
            if j != 0:
                lhsT = make_lhsT.gens[f"h{j}"]
            else:
                h0_full = res
    # taylor + setup pools die here

    # ---------------- phase C: squarings --------------------------------
    uag_out = [pools["dram"].tile([GROUP, 2, HALF, SH], F32,
                                  tag=f"uag_out{h}", name=f"uag_out{h}")
               for h in (0, 1)]
    with tc.tile_pool(name="usq", bufs=1) as usq_pool:
        # the j==0 Horner step above gathered its output but did not build
        # lhsT; do that now from its cur planes
        full = h0_full
        lhsT = make_lhsT(mm_step.cur, "sq_in")
        u32 = None
        for sq in range(S_SQ):
            last = sq == S_SQ - 1
            if not last:
                def evac_sq(nch, mt, ps_r, ps_i):
                    for plane, bank in (("r", ps_r), ("i", ps_i)):
                        split_into(mm_step.cur, bank[:], mt, nch, plane)

                mm_step.cur = new_cur(f"sq{sq}")
                full = mm_step(lhsT, full, f"sq{sq}", evac_sq, gather=True,
                               transpose=True)
                lhsT = make_lhsT.gens[f"sq{sq}"]
            else:
                u32 = {p: usq_pool.tile([P, MT, D], F32, tag=f"u32_{p}",
                                        name=f"u32_{p}")
                       for p in ("r", "i")}
                uts = {p: usq_pool.tile([P, KT, SH], F32, tag=f"uts_{p}",
                                        name=f"uts_{p}")
                       for p in ("r", "i")}

                def evac_u(nch, mt, ps_r, ps_i):
                    csl = slice(nch * HALF, (nch + 1) * HALF)
                    nc.vector.tensor_copy(u32["r"][:, mt, csl], ps_r[:])
                    nc.vector.tensor_copy(u32["i"][:, mt, csl], ps_i[:])

                def u_half_done(h):
                    # transpose this half of U (fp32), bounce + AllGather it
                    for p in ("r", "i"):
                        for mt in range(MT):
                            for c in range(4):
                                ct = h * 4 + c
                                pt = pst_pool.tile(
                                    [P, P], F32, tag="pst32", bufs=2,
                                    name=f"ptu_{p}_{mt}_{ct}")
                                nc.tensor.transpose(
                                    pt[:],
                                    u32[p][:, mt, ct * P:(ct + 1) * P],
                                    idn32[:])
                                nc.vector.tensor_copy(
                                    uts[p][:, ct, mt * P:(mt + 1) * P],
                                    pt[:])
                    uag_in = pools["dram"].tile(
                        [2, HALF, SH], F32, tag=f"uag_in{h}",
                        name=f"uag_in{h}")
                    for pi, p in enumerate(("r", "i")):
                        nc.sync.dma_start(
                            uag_in[pi].rearrange("(kt p) m -> p kt m", p=P),
                            uts[p][:, h * 4:(h + 1) * 4, :])
                    nc.gpsimd.collective_compute(
                        "AllGather", mybir.AluOpType.bypass,
                        replica_groups=RG,
                        ins=[uag_in[:].opt()], outs=[uag_out[h][:].opt()])

                mm_step(lhsT, full, "u", evac_u, gather=False,
                        transpose=False, after_half=u_half_done)

    expm_scope.close()

    # ---------------- phase D: state matmul (fp32r) ---------------------
    with tc.tile_pool(name="state", bufs=1) as st_pool:
        ut = {}
        for pi, p in enumerate(("r", "i")):
            halves = []
            for h in (0, 1):
                t = st_pool.tile([P, 4, D], F32R, tag=f"ut_{p}{h}",
                                 name=f"ut_{p}{h}")
                for r in range(GROUP):
                    nc.scalar.dma_start(
                        t[:, :, r * SH:(r + 1) * SH],
                        uag_out[h][r, pi].rearrange(
                            "(kt p) m -> p kt m", p=P).bitcast(F32R))
                halves.append(t)
            ut[p] = halves
        nut = []
        for h in (0, 1):
            t = st_pool.tile([P, 4, D], F32R, tag=f"ut_n{h}", name=f"nut{h}")
            nc.vector.tensor_scalar_mul(t[:], ut["i"][h][:], -1.0)
            nut.append(t)

        for b1 in range(XB):
            for xch in range(XC // 512):
                xo = slice(xch * 512, (xch + 1) * 512)
                xc = {}
                for p, nm in (("r", "xr"), ("i", "xi")):
                    t = st_pool.tile([P, KT, 512], F32R, tag=f"xc_{p}",
                                     bufs=2, name=f"xc_{p}_{b1}_{xch}")
                    nc.sync.dma_start(t[:], t_in[nm].ap()[b1, :, xo].rearrange(
                        "(kt p) c -> p kt c", p=P).bitcast(F32R))
                    xc[p] = t
                for mt in range(KT):    # 8 m-tiles over output rows
                    msl = slice(mt * P, (mt + 1) * P)
                    ps_r = ps_pool.tile([P, 512], F32, tag="ps",
                                        name=f"ysr_{b1}_{xch}_{mt}")
                    ps_i = ps_pool.tile([P, 512], F32, tag="ps",
                                        name=f"ysi_{b1}_{xch}_{mt}")
                    for bank, passes in (
                            (ps_r, ((ut["r"], xc["r"]), (nut, xc["i"]))),
                            (ps_i, ((ut["r"], xc["i"]), (ut["i"], xc["r"])))):
                        idx = 0
                        for h in (0, 1):
                            for w, m in passes:
                                wt = w[h] if isinstance(w, list) else w[h]
                                for kt in range(4):
                                    nc.tensor.matmul(
                                        ps_r[:] if bank is ps_r else ps_i[:],
                                        wt[:, kt, msl],
                                        m[:, h * 4 + kt, :],
                                        start=(idx == 0), stop=(idx == 15))
                                    idx += 1
                    for pi, bank in ((0, ps_r), (1, ps_i)):
                        ys = io_pool.tile([P, 512], F32, tag="ys", bufs=2,
                                          name=f"ys_{pi}_{b1}_{xch}_{mt}")
                        nc.vector.tensor_copy(ys[:], bank[:])
                        nc.sync.dma_start(
                            t_out.ap()[pi, b1, mt * P:(mt + 1) * P, xo], ys[:])


_CACHED = {}


def _build_program():
    if "nc" in _CACHED:
        return _CACHED["nc"]
    nc = bacc.Bacc("TRN2", target_bir_lowering=False, debug=False,
                   num_devices=NCORES)
    t_in = {}
    for nm, shp in (("xr", [XB, D, XC]), ("xi", [XB, D, XC]),
                    ("ar", [D, D]), ("ai", [D, D]),
                    ("art", [D, D]), ("ait", [D, D]),
                    ("arr", [SH, D]), ("air", [SH, D]),
                    ("artr", [SH, D]), ("aitr", [SH, D]),
                    ("eye", [SH, D])):
        t_in[nm] = nc.dram_tensor(nm, shp, F32, kind="ExternalInput")
    t_in["idn"] = nc.dram_tensor("idn", [P, P], BF16, kind="ExternalInput")
    t_in["idn32"] = nc.dram_tensor("idn32", [P, P], F32, kind="ExternalInput")
    t_out = nc.dram_tensor("y", [2, XB, D, XC], F32, kind="ExternalOutput")

    with tile.TileContext(nc) as tc:
        with tc.tile_pool(name="io", bufs=1) as io_pool, \
             tc.tile_pool(name="ps", bufs=4, space="PSUM") as ps_pool, \
             tc.tile_pool(name="pst", bufs=4, space="PSUM") as pst_pool, \
             tc.tile_pool(name="dram", bufs=1, space="DRAM") as dram_pool:
            pools = {"io": io_pool, "ps": ps_pool,
                     "pst": pst_pool, "dram": dram_pool}
            _emit(nc, tc, t_in, t_out, pools)
    nc.compile()
    _CACHED["nc"] = nc
    return nc


def _in_maps(x_real, x_imag, U_real, U_imag):
    x4r = np.ascontiguousarray(x_real.reshape(2, 2, D, 4 * XC))
    x4i = np.ascontiguousarray(x_imag.reshape(2, 2, D, 4 * XC))
    idn = np.eye(P, dtype=ml_dtypes.bfloat16)
    idn32 = np.eye(P, dtype=np.float32)
    eye_full = np.eye(D, dtype=np.float32)
    maps = []
    for c in range(NCORES):
        k, r = divmod(c, GROUP)
        rows = slice(r * SH, (r + 1) * SH)
        cols = slice(r * XC, (r + 1) * XC)
        ar = np.ascontiguousarray(U_real[k])
        ai = np.ascontiguousarray(U_imag[k])
        art = np.ascontiguousarray(U_real[k].T)
        ait = np.ascontiguousarray(U_imag[k].T)
        maps.append({
            "xr": np.ascontiguousarray(x4r[k][:, :, cols]),
            "xi": np.ascontiguousarray(x4i[k][:, :, cols]),
            "ar": ar, "ai": ai, "art": art, "ait": ait,
            "arr": np.ascontiguousarray(ar[rows]),
            "air": np.ascontiguousarray(ai[rows]),
            "artr": np.ascontiguousarray(art[rows]),
            "aitr": np.ascontiguousarray(ait[rows]),
            "eye": np.ascontiguousarray(eye_full[rows]),
            "idn": idn, "idn32": idn32,
        })
    return maps


def _ensure_device_backend():
    """bass2jax dispatches through the default jax backend; make sure it is
    the neuron/axon one with all 8 cores visible (a harness may have pinned
    jax to cpu for its own reference computation)."""
    import jax

    try:
        devs = jax.devices()
        if devs and devs[0].platform != "cpu" and len(devs) >= NCORES:
            return
    except Exception:
        pass
    for plat in ("axon", "neuron"):
        try:
            jax.config.update("jax_platforms", plat)
            try:
                jax.clear_backends()
            except Exception:
                try:
                    from jax.extend.backend import clear_backends
                    clear_backends()
                except Exception:
                    pass
            devs = jax.devices()
            if devs and len(devs) >= NCORES:
                return
        except Exception:
            continue
    raise RuntimeError("could not find a jax backend with 8 neuron cores")


def kernel(x_real, x_imag, U_real, U_imag, _want_trace=False):
    x_real = np.asarray(x_real, dtype=np.float32).reshape(-1)
    x_imag = np.asarray(x_imag, dtype=np.float32).reshape(-1)
    U_real = np.asarray(U_real, dtype=np.float32)
    U_imag = np.asarray(U_imag, dtype=np.float32)
    _ensure_device_backend()

    nc = _build_program()
    maps = _in_maps(x_real, x_imag, U_real, U_imag)
    res = run_bass_kernel_spmd(nc, maps, core_ids=list(range(NCORES)),
                               trace=_want_trace)
    outs = [res.results[c]["y"] for c in range(NCORES)]

    full = np.empty((2, 2, 2, D, 4, XC), dtype=np.float32)
    for c in range(NCORES):
        k, r = divmod(c, GROUP)
        full[:, k, :, :, r, :] = outs[c]
    y = full.reshape(2, 1 << 24, 1)
    if _want_trace:
        return y, res
    return y
